# revision 26
# baseline (speedup 1.0000x reference)
"""Trainium2 Bass kernel for DiffVAE assm scoring (segment softmax CE loss + acc).

Computation (see reference):
  x_pool = einsum("blh,kh->bk", x_mol_vecs, W_assm)        [32, 448]
  scores[t] = dot(x_pool[batch_idx[t]], cand_vecs[t])      [200000]
  per segment (25 cands): lse, label score, acc flag
  loss = sum(lse - label_score)/32 ; acc = mean(label >= segmax)

Sharding: candidates/segments split evenly across 8 cores (25000 cands =
1000 segments per core, segments kept whole). x_mol_vecs / W_assm
replicated; each core computes x_pool redundantly on-device. Per-core
output is a [128, 16] tile of per-segment partial losses and acc flags,
summed on host.

Device strategy per core:
  - gather one-hot weights are generated ON CHIP: a 102KB batch-index
    table is loaded from HBM, partition-broadcast with one SBUF->SBUF DMA,
    and compared (is_equal) against the per-partition lane id - saves
    ~3.2MB of HBM traffic per core vs. loading the dense one-hot.
  - preamble (PE): x_sum^T via block-one-hot matmul contraction over the
    (b,l) rows (chunks consumed as their DMAs land); x_pool = x_sum @ W^T
    via 4 K-chunk matmuls; replicated to the 4x32 partition groups with
    SBUF->SBUF DMAs.
  - main loop over (seg_block 0..7, cand_slot 0..24):
      DMA cand tile [128 segs, 5 slots, 448] (contiguous 9KB/partition runs)
      PE: xg = onehot^T.T @ x_pool  (float32r: FP22 mult, 1 cyc/row, the
          one-hot side is exact; only x_pool sees the FP22 truncation)
      DVE: fused multiply+reduce (scalar_tensor_tensor accum) -> scores
          column [128,1]
    scores accumulate into [128 segs, 25] tiles; segment softmax stats via
    DVE max-reduce + ACT exp-with-accum-sum + ACT ln; label select via
    one-hot fused multiply+reduce; acc flag via exact is_ge compare.

Engine budget per core (cost-model): DMA ~144us (bound), DVE ~123us,
PE ~46us, ACT ~5us; total ~157us model / ~148us expected on HW (the
model prices the SBUF->SBUF broadcast at HBM rate).
"""

import numpy as np

import concourse.bass as bass
import concourse.tile as tile
from concourse import mybir
from concourse.bass_utils import run_bass_kernel_spmd

# problem constants (hardcoded per harness contract)
B, L, H = 32, 40, 448
S, NCAND = 8000, 25
T = S * NCAND
N_CORES = 8
TC = T // N_CORES          # 25000 candidates per core
SC = S // N_CORES          # 1000 segments per core
NBLK = (SC + 127) // 128   # 8 segment blocks (7 full + 104)
HCH = 112                  # h-chunk for preamble (448 = 4*112)
CCH = 5                    # cand slots per DMA chunk

f32 = mybir.dt.float32
f32r = mybir.dt.float32r
Alu = mybir.AluOpType
Act = mybir.ActivationFunctionType


def _split_multi_waits(nc):
    """This walrus build only encodes a single sem-wait per instruction for
    several instruction classes (CTRL/Drain, S3_LW/ldweights, ...). Keep one
    wait on each instruction and move extras onto preceding NOPs issued on
    the same engine (engine queues are FIFO, so ordering is preserved)."""
    f = nc.m.functions[0]

    def make_nop(engine):
        nw = nc.engines[engine].nop().ins
        for b2 in f.blocks:
            if nw in b2.instructions:
                b2.instructions.remove(nw)
        return nw

    for bb in f.blocks:
        multi = [i for i in bb.instructions
                 if i.sync_info and len(i.sync_info.on_wait) > 1]
        for d in multi:
            waits = list(d.sync_info.on_wait)
            extra, keep = waits[:-1], waits[-1:]
            nops = []
            for w in extra:
                nw = make_nop(d.engine)
                nw.sync_info = mybir.SyncInfo(on_wait=[w], on_update=[])
                nops.append(nw)
            d.sync_info = mybir.SyncInfo(on_wait=keep,
                                         on_update=list(d.sync_info.on_update))
            idx = bb.instructions.index(d)
            bb.instructions[idx:idx] = nops


def build_bass():
    nc = bass.Bass("TRN2", target_bir_lowering=False, debug=False)

    cand = nc.dram_tensor("cand", [TC, H], f32, kind="ExternalInput").ap()
    xf = nc.dram_tensor("xf", [B * L, H], f32, kind="ExternalInput").ap()
    wt = nc.dram_tensor("wt", [H, H], f32, kind="ExternalInput").ap()
    ohb = nc.dram_tensor("ohb", [128, 10, B], f32, kind="ExternalInput").ap()
    bidxq = nc.dram_tensor("bidxq", [4, 6400], f32, kind="ExternalInput").ap()
    qmod = nc.dram_tensor("qmod", [128, 1], f32, kind="ExternalInput").ap()
    loh = nc.dram_tensor("loh", [128, NBLK, NCAND], f32, kind="ExternalInput").ap()
    out = nc.dram_tensor("out", [128, 2 * NBLK], f32, kind="ExternalOutput").ap()

    with tile.TileContext(nc) as tc:
        with (
            tc.tile_pool(name="singles", bufs=1) as singles,
            tc.tile_pool(name="pre_ps", bufs=1, space="PSUM") as pre_ps,
            tc.tile_pool(name="xg_ps", bufs=3, space="PSUM") as xg_ps,
            tc.tile_pool(name="cand_p", bufs=8) as cand_p,
            tc.tile_pool(name="ttro", bufs=2) as ttro_p,
            tc.tile_pool(name="sc_p", bufs=3) as sc_p,
            tc.tile_pool(name="small", bufs=10) as small,
            tc.tile_pool(name="ep", bufs=2) as ep,
        ):
            # ---- load replicated operands (preamble gates first, then
            #      first candidate block so DMA streams immediately) ----
            ohb_sb = singles.tile([128, 10, B], f32)
            nc.sync.dma_start(ohb_sb, ohb)
            # xf in chunks so preamble matmuls start as soon as each lands
            xf_sb = singles.tile([128, 10, H], f32)
            xf_r = xf.rearrange("(n p) h -> p n h", p=128)
            for jk in range(10):
                nc.sync.dma_start(xf_sb[:, jk, :], xf_r[:, jk, :])
            wt_sb = singles.tile([HCH, 4, H], f32)
            nc.sync.dma_start(wt_sb, wt.rearrange("(n p) k -> p n k", p=HCH))

            loh_sb = singles.tile([128, NBLK, NCAND], f32)
            cand_r = cand.rearrange("(s c) h -> s c h", c=NCAND)

            # one-hot gather weights generated on-chip: tiny batch-index
            # table from HBM, partition-broadcast via SBUF->SBUF DMA, then
            # a single DVE is_equal against the per-partition lane id
            bidx_sb = singles.tile([4, 6400], f32)
            nc.sync.dma_start(bidx_sb, bidxq)
            qmod_sb = singles.tile([128, 1], f32)
            nc.sync.dma_start(qmod_sb, qmod)
            bc_sb = singles.tile([128, 6400], f32)
            bap = bidx_sb[:]
            nc.sync.dma_start(bc_sb, bass.AP(tensor=bap.tensor, offset=bap.offset,
                                             ap=[bap.ap[0], [0, 32], bap.ap[1]]))
            oht_sb = singles.tile([128, 6400], f32r)
            nc.vector.tensor_scalar(out=oht_sb, in0=bc_sb, scalar1=qmod_sb[:],
                                    scalar2=None, op0=Alu.is_equal)

            def issue_oht(k):
                nc.sync.dma_start(loh_sb[:, k, :], loh[:, k, :])

            border = list(range(NBLK))

            def issue_cand(k, last=False):
                rows = min(128, SC - k * 128)
                # finer trailing chunks on the last block shorten the
                # compute tail after the final DMA byte lands
                sizes = [5, 5, 5, 5, 5]
                cts = []
                c0 = 0
                for n in sizes:
                    ct = cand_p.tile([128, CCH, H], f32, tag="ct", name="ct")
                    nc.sync.dma_start(
                        ct[:rows, :n, :],
                        cand_r[k * 128:k * 128 + rows, c0:c0 + n, :],
                    )
                    cts.append((ct, c0, n))
                    c0 += n
                return cts

            issue_oht(border[0])
            pending = issue_cand(border[0])

            out_sb = singles.tile([128, 2 * NBLK], f32)
            nc.vector.memset(out_sb, 0.0)

            # ---- preamble: x_sum^T then x_pool, replicated to 4 row groups ----
            # jk-outer so each xf chunk is consumed as it lands
            xsT_sb = singles.tile([HCH, 4, B], f32)
            pss = [pre_ps.tile([HCH, B], f32, tag=f"ps{jh}", name=f"ps{jh}")
                   for jh in range(4)]
            for jk in range(10):
                for jh in range(4):
                    nc.tensor.matmul(
                        pss[jh],
                        lhsT=xf_sb[:, jk, jh * HCH:(jh + 1) * HCH],
                        rhs=ohb_sb[:, jk, :],
                        start=(jk == 0), stop=(jk == 9),
                    )
            for jh in range(4):
                nc.scalar.copy(xsT_sb[:, jh, :], pss[jh])

            pool_ps = pre_ps.tile([32, H], f32, tag="poolps")
            for jh in range(4):
                nc.tensor.matmul(
                    pool_ps,
                    lhsT=xsT_sb[:, jh, :],
                    rhs=wt_sb[:, jh, :],
                    start=(jh == 0), stop=(jh == 3),
                )
            xpool4_sb = singles.tile([128, H], f32r)
            nc.scalar.copy(xpool4_sb[0:32, :], pool_ps)
            for q in range(1, 4):
                nc.sync.dma_start(xpool4_sb[32 * q:32 * q + 32, :],
                                  xpool4_sb[0:32, :])

            # ---- main loop ----
            for kord in range(NBLK):
                k = border[kord]
                rows = min(128, SC - k * 128)
                sc = sc_p.tile([128, NCAND], f32)
                cts = pending
                if kord + 1 < NBLK:
                    knext = border[kord + 1]
                    issue_oht(knext)
                    pending = issue_cand(knext, last=(kord + 1 == NBLK - 1))
                for ct, c0, n in cts:
                    for ci in range(n):
                        c = c0 + ci
                        g = k * NCAND + c
                        q, r = divmod(g, 50)
                        xg = xg_ps.tile([128, H], f32)
                        nc.tensor.matmul(
                            xg[:rows],
                            lhsT=oht_sb[32 * q:32 * q + 32,
                                        r * 128:r * 128 + rows],
                            rhs=xpool4_sb[32 * q:32 * q + 32, :],
                            start=True, stop=True,
                            tile_position=(32 * q, 0),
                        )
                        ttro = ttro_p.tile([128, H], f32)
                        nc.vector.scalar_tensor_tensor(
                            out=ttro[:rows],
                            in0=ct[:rows, ci, :],
                            scalar=1.0,
                            in1=xg[:rows],
                            op0=Alu.mult, op1=Alu.mult,
                            accum_out=sc[:rows, c:c + 1],
                        )
                # segment softmax stats for this block
                nm = small.tile([128, 1], f32)
                nc.vector.tensor_reduce(nm[:rows], sc[:rows, :],
                                        axis=mybir.AxisListType.X,
                                        op=Alu.max, negate=True)
                m = small.tile([128, 1], f32)
                nc.vector.tensor_scalar_mul(m[:rows], nm[:rows], -1.0)
                e = ep.tile([128, NCAND], f32)
                ssum = small.tile([128, 1], f32)
                nc.scalar.activation(e[:rows], sc[:rows, :], func=Act.Exp,
                                     bias=nm[:rows], scale=1.0,
                                     accum_out=ssum[:rows])
                ls = small.tile([128, 1], f32)
                nc.scalar.activation(ls[:rows], ssum[:rows], func=Act.Ln)
                lse = small.tile([128, 1], f32)
                nc.vector.tensor_sub(lse[:rows], ls[:rows], nm[:rows])
                lab = small.tile([128, 1], f32)
                ttro2 = ep.tile([128, NCAND], f32)
                nc.vector.scalar_tensor_tensor(
                    out=ttro2[:rows],
                    in0=sc[:rows, :],
                    scalar=1.0,
                    in1=loh_sb[:rows, k, :],
                    op0=Alu.mult, op1=Alu.mult,
                    accum_out=lab[:rows],
                )
                nc.vector.tensor_sub(out_sb[:rows, k:k + 1], lse[:rows], lab[:rows])
                nc.vector.tensor_tensor(out_sb[:rows, NBLK + k:NBLK + k + 1],
                                        lab[:rows], m[:rows], op=Alu.is_ge)

            nc.sync.dma_start(out, out_sb)

    _split_multi_waits(nc)
    return nc


def make_inputs(x_mol_vecs, cand_vecs, W_assm, batch_idx, label_in_seg):
    """Host-side shard + index preprocessing. Returns per-core input maps."""
    x = np.asarray(x_mol_vecs, np.float32).reshape(B * L, H)
    cand = np.asarray(cand_vecs, np.float32)
    W = np.asarray(W_assm, np.float32)
    bi = np.asarray(batch_idx).astype(np.int64)
    lab = np.asarray(label_in_seg).astype(np.int64)

    wt = np.ascontiguousarray(W.T)
    qmod = (np.arange(128) % 32).astype(np.float32).reshape(128, 1)

    # block one-hot for pooling over (b,l) rows: row r -> batch r//L
    r = np.arange(B * L)
    ohb = np.zeros((B * L, B), np.float32)
    ohb[r, r // L] = 1.0
    ohb = np.ascontiguousarray(ohb.reshape(10, 128, B).transpose(1, 0, 2))

    in_maps = []
    for core in range(N_CORES):
        s0 = core * SC
        bi_c = bi[core * TC:(core + 1) * TC].reshape(SC, NCAND)
        lab_c = lab[s0:s0 + SC]

        bidxq = np.zeros((4, 6400), np.float32)
        for g in range(NBLK * NCAND):
            k, cc = divmod(g, NCAND)
            q, rr = divmod(g, 50)
            rows = min(128, SC - k * 128)
            segs = np.arange(rows) + k * 128
            bidxq[q, rr * 128 + np.arange(rows)] = bi_c[segs, cc]

        loh = np.zeros((128, NBLK, NCAND), np.float32)
        segs = np.arange(SC)
        loh[segs % 128, segs // 128, lab_c] = 1.0

        in_maps.append({
            "cand": np.ascontiguousarray(cand[core * TC:(core + 1) * TC]),
            "xf": x,
            "wt": wt,
            "ohb": ohb,
            "bidxq": bidxq,
            "qmod": qmod,
            "loh": loh,
        })
    return in_maps


_NC_CACHE = None


def kernel(x_mol_vecs, cand_vecs, W_assm, batch_idx, label_in_seg,
           ncand=NCAND, num_segments=S, **_ignored):
    global _NC_CACHE
    assert int(ncand) == NCAND and int(num_segments) == S

    in_maps = make_inputs(x_mol_vecs, cand_vecs, W_assm, batch_idx, label_in_seg)
    if _NC_CACHE is None:
        _NC_CACHE = build_bass()
    res = run_bass_kernel_spmd(_NC_CACHE, in_maps, core_ids=list(range(N_CORES)))

    loss_sum = 0.0
    acc_sum = 0.0
    for core in range(N_CORES):
        o = res.results[core]["out"]
        loss_sum += float(o[:, :NBLK].sum(dtype=np.float64))
        acc_sum += float(o[:, NBLK:].sum(dtype=np.float64))
    loss = np.float32(loss_sum / B)
    acc = np.float32(acc_sum / S)
    return loss, acc
